# revision 40
# baseline (speedup 1.0000x reference)
"""Trainium2 Bass kernel for nn_AttentionNeuralOperator (dense_transformer).

Strategy: the per-execute dispatch cost on this platform scales with
(#cores) and (#args x #cores); bytes are comparatively cheap. So all 64
sequences run on ONE NeuronCore, and inputs ship as two packed blobs
(fp32 + bf16) -> 2 input args + 1 output arg.

Per sequence (c=128 channels, L=576=24x24 tokens, 2 heads, d_qk=64, d_v=128):
    qk  = qk_wT.T @ x            -> q,k in (d, L) layout        [PE]
    vT  = x.T @ v_wT             -> v in (L, d) layout, bf16    [PE]
    scoresT = k_h.T q_h          -> (m-part, l-free)            [PE]
    expT = exp(scoresT)          -> bf16                        [ACT]
    expT *= expbiasT             (multiplicative position bias) [DVE bf16]
    sums = ones.T @ expT into one PSUM row -> reciprocal [DVE] ->
           broadcast back via a DRAM round-trip
    out_h = vT_h.T @ expT_h      -> (d-part, l-free)            [PE]
    normalized by broadcast(1/sums) [DVE], then 1x1-conv MLP (bf16
    weights) with exact GELU [PE+ACT].
Sequences are processed in groups of 16: attention for the group, then the
MLP for the group, so the ACT engine switches its Exp/Gelu table sets only
twice per group. Position bias is evaluated on host on the 47x47 distinct
(dy,dx) grid, expanded, exp'd, and shipped in the bf16 blob.
Matmuls run in float32r (tf32-class) / bf16; PSUM accumulation is fp32.
NOTE: the Pool engine (gpsimd) cannot access PSUM on TRN2 - all
PSUM-touching elementwise work lives on DVE/ACT.
"""
import sys
sys.path.insert(0, "/opt/trn_rl_repo")
import numpy as np
import ml_dtypes

import concourse.bass as bass
import concourse.tile as tile
from concourse.tile import add_dep_helper
from concourse import bacc, mybir
from concourse.bass_utils import run_bass_kernel_spmd

P = 128
HEADS = 2
B, S, C, HH, WW = 2, 32, 128, 24, 24
L = HH * WW            # 576
LP = 640               # m padded to 5*128
NSEQ = B * S           # 64
NCORES = 1
SEQ_PER_CORE = NSEQ // NCORES  # 64
GROUP = 16             # seqs per phase-group (ACT table switches: 2/group)
HID = 256
OUT_CH = 128
QKD = C // HEADS       # 64
VD = HID // HEADS      # 128
NCH = LP // P          # 5 m-chunks
PIECE = 288            # l-piece (>=256 keeps float32r at full rate)
F32 = mybir.dt.float32
F32R = mybir.dt.float32r
BF16 = mybir.dt.bfloat16
BF = ml_dtypes.bfloat16

# fp32 blob layout (element offsets)
SEQ_ELEMS = C * L                         # 73728
NX = NSEQ * SEQ_ELEMS                     # 4718592
OFF_X = 0
OFF_QKW = OFF_X + NX                      # [C, 2C]
OFF_VW = OFF_QKW + C * 2 * C              # [C, HID]
OFF_B1 = OFF_VW + C * HID                 # [P, 2]
OFF_B2 = OFF_B1 + P * 2                   # [P, 1]
NBLOB = OFF_B2 + P

# bf16 blob layout (element offsets)
BOFF_BIAS = 0                             # [P, NCH, HEADS*L]
BOFF_ONES = BOFF_BIAS + P * NCH * HEADS * L   # [P, 32]
BOFF_W1 = BOFF_ONES + P * 32              # [P, 2, HID]
BOFF_W2 = BOFF_W1 + P * 2 * HID           # [P, 2, OUT_CH]
NBLOB_BF = BOFF_W2 + P * 2 * OUT_CH


def _log_cpb_np(h, w, w1, b1, w2):
    """Host fp32 mirror of the reference CPB MLP, on the 47x47 delta grid."""
    dy = np.arange(-(h - 1), h, dtype=np.float32)
    dx = np.arange(-(w - 1), w, dtype=np.float32)
    rel = np.stack(np.meshgrid(dy, dx, indexing="ij"), axis=-1)     # (2h-1, 2w-1, 2)
    denom = np.array([max(h - 1, 1), max(w - 1, 1)], dtype=np.float32)
    rel = rel / denom * np.float32(8.0)
    rel = np.sign(rel) * np.log2(np.float32(1.0) + np.abs(rel)) / np.float32(np.log2(8.0))
    hid_act = np.maximum(rel @ w1.T + b1, np.float32(0.0))          # (2h-1, 2w-1, c)
    tab = (hid_act @ w2.T).astype(np.float32)                       # (2h-1, 2w-1, heads)
    yl = np.repeat(np.arange(h), w)
    xl = np.tile(np.arange(w), h)
    DY = yl[:, None] - yl[None, :] + (h - 1)                        # (L, L)
    DX = xl[:, None] - xl[None, :] + (w - 1)
    return tab[DY, DX].transpose(2, 0, 1)                           # (heads, L, L)


def _prep_aux(qk_w, v_w, cpb_w1, cpb_b1, cpb_w2, sa_bias, mlp_w1, mlp_b1, mlp_w2, mlp_b2):
    scale = np.float32(1.0 / np.sqrt(QKD))
    qkwT = np.ascontiguousarray(qk_w.T).astype(np.float32).copy()   # (c, 2c)
    qkwT[:, :C] *= scale                                            # fold attn scale into q
    vwT = np.ascontiguousarray(v_w.T).astype(np.float32)            # (c, hid)

    bias = _log_cpb_np(HH, WW, cpb_w1, cpb_b1, cpb_w2)              # (heads, L, L)
    # multiplicative bias: exp(s+b) = exp(s)*exp(b); padded m-rows get 0 so
    # they vanish from the softmax sums and the attn@v contraction
    ebias = np.zeros((HEADS, LP, L), dtype=np.float32)
    ebias[:, :L, :] = np.exp(bias.transpose(0, 2, 1))               # [h, m, l]
    biasT_sb = np.empty((P, NCH, HEADS * L), dtype=np.float32)
    for ch in range(NCH):
        for h in range(HEADS):
            biasT_sb[:, ch, h * L:(h + 1) * L] = ebias[h, ch * P:(ch + 1) * P, :]

    w1T = np.empty((P, 2, HID), dtype=np.float32)                   # [p, kt, o]
    for kt in range(2):
        w1T[:, kt, :] = mlp_w1[:, kt * P:(kt + 1) * P].T
    w2T = np.empty((P, 2, OUT_CH), dtype=np.float32)
    for kt in range(2):
        w2T[:, kt, :] = mlp_w2[:, kt * P:(kt + 1) * P].T
    b1c = (mlp_w1 @ sa_bias.reshape(-1) + mlp_b1).astype(np.float32).reshape(2, P).T.copy()  # (128, 2)
    b2c = mlp_b2.astype(np.float32).reshape(P, 1).copy()
    return {
        "qkwT": qkwT, "vwT": vwT, "biasT": biasT_sb,
        "w1T": w1T, "w2T": w2T, "b1c": b1c, "b2c": b2c,
    }


def _pack_blobs(x, aux):
    """fp32 blob: x + projection weights; bf16 blob: exp-bias, ones, MLP."""
    blob = np.empty(NBLOB, dtype=np.float32)
    blob[OFF_X:OFF_X + NX] = np.ascontiguousarray(x, dtype=np.float32).reshape(-1)
    for off, name in ((OFF_QKW, "qkwT"), (OFF_VW, "vwT"),
                      (OFF_B1, "b1c"), (OFF_B2, "b2c")):
        a = aux[name].reshape(-1)
        blob[off:off + a.size] = a
    blob_bf = np.empty(NBLOB_BF, dtype=BF)
    blob_bf[BOFF_BIAS:BOFF_ONES] = aux["biasT"].reshape(-1).astype(BF)
    blob_bf[BOFF_ONES:BOFF_W1] = np.ones(P * 32, dtype=BF)
    blob_bf[BOFF_W1:BOFF_W2] = aux["w1T"].reshape(-1).astype(BF)
    blob_bf[BOFF_W2:] = aux["w2T"].reshape(-1).astype(BF)
    return blob, blob_bf


def _gap(ap):
    """View a (128, 1024) psum tile as (128, 2, 288): pieces at [0:288], [512:800]."""
    return ap.rearrange("p (g c) -> p g c", c=512)[:, :, :PIECE]


def _pieces(ap576):
    """View a contiguous (128, 576) AP as (128, 2, 288)."""
    return ap576.rearrange("p (g c) -> p g c", c=PIECE)


def build_kernel(seqs=SEQ_PER_CORE, num_devices=NCORES, repeat=1, skip=()):
    nc = bacc.Bacc("TRN2", target_bir_lowering=False, debug=False,
                   num_devices=num_devices)
    blob_d = nc.dram_tensor("blob", [NBLOB], F32R, kind="ExternalInput").ap()
    blobb_d = nc.dram_tensor("blobb", [NBLOB_BF], BF16, kind="ExternalInput").ap()
    y_d = nc.dram_tensor("y", [seqs, OUT_CH, L], F32, kind="ExternalOutput").ap()

    x_d = blob_d[OFF_X:OFF_X + seqs * SEQ_ELEMS].rearrange(
        "(t c l) -> t c l", c=C, l=L)
    qkwT_d = blob_d[OFF_QKW:OFF_VW].rearrange("(c o) -> c o", o=2 * C)
    vwT_d = blob_d[OFF_VW:OFF_B1].rearrange("(c o) -> c o", o=HID)
    b1c_d = blob_d[OFF_B1:OFF_B2].rearrange("(p k) -> p k", k=2).bitcast(F32)
    b2c_d = blob_d[OFF_B2:NBLOB].rearrange("(p k) -> p k", k=1).bitcast(F32)
    biasT_d = blobb_d[BOFF_BIAS:BOFF_ONES].rearrange(
        "(p n hl) -> p n hl", n=NCH, hl=HEADS * L)
    ones_d = blobb_d[BOFF_ONES:BOFF_W1].rearrange("(p k) -> p k", k=32)
    w1T_d = blobb_d[BOFF_W1:BOFF_W2].rearrange("(p k o) -> p k o", k=2, o=HID)
    w2T_d = blobb_d[BOFF_W2:].rearrange("(p k o) -> p k o", k=2, o=OUT_CH)

    EXP = mybir.ActivationFunctionType.Exp
    GELU = mybir.ActivationFunctionType.Gelu
    MULT = mybir.AluOpType.mult

    ngroups = (seqs + GROUP - 1) // GROUP

    with tile.TileContext(nc) as tc:
        with (
            tc.tile_pool(name="const", bufs=1) as cpool,
            tc.tile_pool(name="xin", bufs=3) as xpool,
            tc.tile_pool(name="qk", bufs=3) as qkpool,
            tc.tile_pool(name="vt", bufs=3) as vtpool,
            tc.tile_pool(name="ex", bufs=3) as expool,
            tc.tile_pool(name="sm", bufs=2) as smpool,
            tc.tile_pool(name="xb", bufs=2) as xbpool,
            tc.tile_pool(name="drb", bufs=2, space="DRAM") as drpool,
            tc.tile_pool(name="acts", bufs=GROUP) as apool,
            tc.tile_pool(name="mlp", bufs=2) as mpool,
            tc.tile_pool(name="psg", bufs=2, space="PSUM") as psg,
            tc.tile_pool(name="ps1", bufs=4, space="PSUM") as ps1,
        ):
            qkwT = cpool.tile([C, 2 * C], F32R)
            vwT = cpool.tile([C, HID], F32R)
            biasT = cpool.tile([P, NCH, HEADS * L], BF16)
            ones32 = cpool.tile([P, 32], BF16)
            w1T = cpool.tile([P, 2, HID], BF16)
            w2T = cpool.tile([P, 2, OUT_CH], BF16)
            b1c = cpool.tile([P, 2], F32)
            b2c = cpool.tile([P, 1], F32)
            for sb_t, dr in ((qkwT, qkwT_d), (vwT, vwT_d), (biasT, biasT_d),
                             (ones32, ones_d), (w1T, w1T_d), (w2T, w2T_d),
                             (b1c, b1c_d), (b2c, b2c_d)):
                nc.sync.dma_start(sb_t[:], dr[:])

            _last_exp = [None]
            _last_gelu = [None]
            # repeat>1 is a timing-only mode: reps chain serially through pools
            for _rep in range(repeat):
              for grp in range(ngroups):
                t0 = grp * GROUP
                gseqs = range(t0, min(t0 + GROUP, seqs))
                a_tiles = {}
                # ---------------- Phase A: attention ----------------
                for t in gseqs:
                    xt = xpool.tile([C, LP], F32R)
                    nc.sync.dma_start(xt[:, :L], x_d[t])
                    nc.gpsimd.memset(xt[:, L:].bitcast(F32), 0.0)

                    # qk projection: q rows (h0 d, h1 d), k rows (h0 d, h1 d)
                    q_sb = qkpool.tile([P, L], F32R, tag="q")
                    k_sb = qkpool.tile([P, LP], F32R, tag="k")
                    for mt, dst in ((0, q_sb[:, :L]), (1, k_sb[:, :L])):
                        pqk = psg.tile([P, 1024], F32, tag="g")
                        for lh in range(2):
                            nc.tensor.matmul(
                                _gap(pqk[:])[:, lh],
                                qkwT[:, mt * P:(mt + 1) * P],
                                xt[:, lh * PIECE:(lh + 1) * PIECE],
                                start=True, stop=True)
                        nc.vector.tensor_copy(_pieces(dst), _gap(pqk[:]))
                    nc.gpsimd.memset(k_sb[:, L:].bitcast(F32), 0.0)

                    # vT: (m-part chunks, 256 = both heads' d), bf16
                    vt_sb = vtpool.tile([P, NCH, HID], BF16)
                    for ch in range(NCH):
                        pv = ps1.tile([P, PIECE], F32, tag="s")
                        nc.tensor.matmul(pv[:, :HID], xt[:, ch * P:(ch + 1) * P], vwT[:],
                                         start=True, stop=True)
                        nc.vector.tensor_copy(vt_sb[:, ch, :], pv[:, :HID])

                    # scores + exp (bf16) per (chunk, head); bias-mult per chunk
                    ex_sb = expool.tile([P, NCH, HEADS * L], F32R)
                    a_sb = apool.tile([P, 2, L], F32R)
                    # head-major: each head's full softmax chain (scores ->
                    # exp -> bias-mult -> sums -> recip -> broadcast -> attnv
                    # -> norm) runs to completion while the other head's
                    # scores fill the PE pipe. The sums matmul then waits
                    # only on its own head's 5 Pool multiplies instead of 9
                    # of 10, and the normalize waits only on its own half
                    # of the reciprocal broadcast.
                    for h in range(HEADS):
                        for ch in range(NCH):
                            psc = psg.tile([P, 1024], F32, tag="g")
                            for lh in range(2):
                                nc.tensor.matmul(
                                    _gap(psc[:])[:, lh],
                                    k_sb[h * QKD:(h + 1) * QKD, ch * P:(ch + 1) * P],
                                    q_sb[h * QKD:(h + 1) * QKD, lh * PIECE:(lh + 1) * PIECE],
                                    start=True, stop=True)
                            e = nc.scalar.activation(
                                _pieces(ex_sb[:, ch, h * L:(h + 1) * L]), _gap(psc[:]), EXP)
                            if _last_gelu[0] is not None:
                                add_dep_helper(_last_gelu[0].ins, e.ins, sync=False,
                                               reason="ACT table set phase order")
                                _last_gelu[0] = None
                            _last_exp[0] = e
                            nc.gpsimd.tensor_tensor(
                                ex_sb[:, ch, h * L:(h + 1) * L],
                                ex_sb[:, ch, h * L:(h + 1) * L],
                                biasT[:, ch, h * L:(h + 1) * L], MULT)

                        # this head's denominators
                        xb = xbpool.tile([P, 2 * PIECE], F32)
                        inv_sb = smpool.tile([1, 2 * PIECE], F32, tag="inv")
                        for lh in range(2):
                            pss = ps1.tile([P, PIECE], F32, tag="s")
                            for ch in range(NCH):
                                nc.tensor.matmul(
                                    pss[0:1, :], ones32[:, 0:1],
                                    ex_sb[:, ch, h * L + lh * PIECE: h * L + (lh + 1) * PIECE],
                                    start=(ch == 0), stop=(ch == NCH - 1))
                            nc.vector.reciprocal(
                                inv_sb[:, lh * PIECE:(lh + 1) * PIECE], pss[0:1, :])
                        inv_dr = drpool.tile([1, 2 * PIECE], F32)
                        nc.sync.dma_start(inv_dr[:], inv_sb[:])
                        nc.sync.dma_start(xb[:], inv_dr[:].to_broadcast((P, 2 * PIECE)))

                        # this head's attention output + normalize
                        pms = [ps1.tile([P, PIECE], F32, tag="s", name=f"pm{_i}") for _i in range(2)]
                        for ch in range(NCH):
                            for lh in range(2):
                                nc.tensor.matmul(
                                    pms[lh][:], vt_sb[:, ch, h * VD:(h + 1) * VD],
                                    ex_sb[:, ch, h * L + lh * PIECE: h * L + (lh + 1) * PIECE],
                                    start=(ch == 0), stop=(ch == NCH - 1))
                        for lh in range(2):
                            nc.vector.tensor_tensor(
                                a_sb[:, h, lh * PIECE:(lh + 1) * PIECE], pms[lh][:],
                                xb[:, lh * PIECE:(lh + 1) * PIECE], MULT)
                    a_tiles[t] = a_sb

                # ---------------- Phase B: MLP ----------------
                for t in ([] if "mlp" in skip else gseqs):
                    a_sb = a_tiles[t]
                    g_sb = mpool.tile([P, 2, L], BF16, tag="g")
                    for mt in range(2):
                        py1 = psg.tile([P, 1024], F32, tag="g")
                        for lh in range(2):
                            for kt in range(2):
                                nc.tensor.matmul(
                                    _gap(py1[:])[:, lh],
                                    w1T[:, kt, mt * P:(mt + 1) * P],
                                    a_sb[:, kt, lh * PIECE:(lh + 1) * PIECE],
                                    start=(kt == 0), stop=(kt == 1))
                        _g = nc.scalar.activation(
                            _pieces(g_sb[:, mt, :]), _gap(py1[:]), GELU,
                            bias=b1c[:, mt:mt + 1], scale=1.0)
                        if _last_exp[0] is not None:
                            add_dep_helper(_last_exp[0].ins, _g.ins, sync=False,
                                           reason="ACT table set phase order")
                            _last_exp[0] = None
                        _last_gelu[0] = _g
                    y_sb = mpool.tile([OUT_CH, L], F32, tag="y")
                    for lh in range(2):
                        py2 = ps1.tile([P, PIECE], F32, tag="s")
                        for kt in range(2):
                            nc.tensor.matmul(
                                py2[:], w2T[:, kt, :],
                                g_sb[:, kt, lh * PIECE:(lh + 1) * PIECE],
                                start=(kt == 0), stop=(kt == 1))
                        nc.vector.tensor_scalar_add(
                            y_sb[:, lh * PIECE:(lh + 1) * PIECE], py2[:], b2c[:, 0:1])
                    nc.sync.dma_start(y_d[t], y_sb[:])
                if "mlp" in skip:
                    for t in gseqs:
                        nc.sync.dma_start(y_d[t].rearrange("c l -> c l"),
                                          a_tiles[t][:, 0, :].bitcast(F32))
    nc.compile()
    return nc


_CACHED = {}


def _get_nc():
    if "nc" not in _CACHED:
        _CACHED["nc"] = build_kernel()
    return _CACHED["nc"]


def make_in_maps(x, aux):
    xr = np.ascontiguousarray(x.reshape(NSEQ, C, L), dtype=np.float32)
    in_maps = []
    for i in range(NCORES):
        xs = xr[i * SEQ_PER_CORE:(i + 1) * SEQ_PER_CORE]
        blob, blob_bf = _pack_blobs(xs, aux)
        in_maps.append({"blob": blob, "blobb": blob_bf})
    return in_maps


def kernel(x, qk_w, v_w, cpb_w1, cpb_b1, cpb_w2, sa_bias,
           mlp_w1, mlp_b1, mlp_w2, mlp_b2):
    x = np.asarray(x)
    aux = _prep_aux(np.asarray(qk_w), np.asarray(v_w), np.asarray(cpb_w1),
                    np.asarray(cpb_b1), np.asarray(cpb_w2), np.asarray(sa_bias),
                    np.asarray(mlp_w1), np.asarray(mlp_b1), np.asarray(mlp_w2),
                    np.asarray(mlp_b2))
    nc = _get_nc()
    in_maps = make_in_maps(x, aux)
    res = run_bass_kernel_spmd(nc, in_maps, core_ids=list(range(NCORES)))
    y = np.concatenate([res.results[i]["y"] for i in range(NCORES)], axis=0)
    return y.reshape(B, S, OUT_CH, HH, WW)


if __name__ == "__main__":
    import reference
    inputs = reference.setup_inputs()
    inputs = {k: np.asarray(v) for k, v in inputs.items()}
    out = kernel(**inputs)
    exp = np.asarray(reference.reference(**reference.setup_inputs()))
    err = np.abs(out - exp).max() / np.abs(exp).max()
    print("Relative error:", err)


# revision 41
# speedup vs baseline: 1.3913x; 1.3913x over previous
"""Trainium2 Bass kernel for nn_AttentionNeuralOperator (dense_transformer).

Strategy: the per-execute dispatch cost on this platform scales with
(#cores) and (#args x #cores); bytes are comparatively cheap. So all 64
sequences run on ONE NeuronCore, and inputs ship as two packed blobs
(fp32 + bf16) -> 2 input args + 1 output arg.

Per sequence (c=128 channels, L=576=24x24 tokens, 2 heads, d_qk=64, d_v=128):
    qk  = qk_wT.T @ x            -> q,k in (d, L) layout        [PE]
    vT  = x.T @ v_wT             -> v in (L, d) layout, bf16    [PE]
    scoresT = k_h.T q_h          -> (m-part, l-free)            [PE]
    expT = exp(scoresT)          -> bf16                        [ACT]
    expT *= expbiasT             (multiplicative position bias) [DVE bf16]
    sums = ones.T @ expT into one PSUM row -> reciprocal [DVE] ->
           broadcast back via a DRAM round-trip
    out_h = vT_h.T @ expT_h      -> (d-part, l-free)            [PE]
    normalized by broadcast(1/sums) [DVE], then 1x1-conv MLP (bf16
    weights) with exact GELU [PE+ACT].
Sequences are processed in groups of 16: attention for the group, then the
MLP for the group, so the ACT engine switches its Exp/Gelu table sets only
twice per group. Position bias is evaluated on host on the 47x47 distinct
(dy,dx) grid, expanded, exp'd, and shipped in the bf16 blob.
Matmuls run in float32r (tf32-class) / bf16; PSUM accumulation is fp32.
NOTE: the Pool engine (gpsimd) cannot access PSUM on TRN2 - all
PSUM-touching elementwise work lives on DVE/ACT.
"""
import sys
sys.path.insert(0, "/opt/trn_rl_repo")
import numpy as np
import ml_dtypes

import concourse.bass as bass
import concourse.tile as tile
from concourse.tile import add_dep_helper
from concourse import bacc, mybir
from concourse.bass_utils import run_bass_kernel_spmd

P = 128
HEADS = 2
B, S, C, HH, WW = 2, 32, 128, 24, 24
L = HH * WW            # 576
LP = 640               # m padded to 5*128
NSEQ = B * S           # 64
NCORES = 1
SEQ_PER_CORE = NSEQ // NCORES  # 64
GROUP = 16             # seqs per phase-group (ACT table switches: 2/group)
HID = 256
OUT_CH = 128
QKD = C // HEADS       # 64
VD = HID // HEADS      # 128
NCH = LP // P          # 5 m-chunks
PIECE = 288            # l-piece (>=256 keeps float32r at full rate)
F32 = mybir.dt.float32
F32R = mybir.dt.float32r
BF16 = mybir.dt.bfloat16
BF = ml_dtypes.bfloat16

# fp32 blob layout (element offsets)
SEQ_ELEMS = C * L                         # 73728
NX = NSEQ * SEQ_ELEMS                     # 4718592
OFF_X = 0
OFF_QKW = OFF_X + NX                      # [C, 2C]
OFF_VW = OFF_QKW + C * 2 * C              # [C, HID]
OFF_B1 = OFF_VW + C * HID                 # [P, 2]
OFF_B2 = OFF_B1 + P * 2                   # [P, 1]
NBLOB = OFF_B2 + P

# bf16 blob layout (element offsets)
BOFF_BIAS = 0                             # [P, NCH, HEADS*L]
BOFF_ONES = BOFF_BIAS + P * NCH * HEADS * L   # [P, 32]
BOFF_W1 = BOFF_ONES + P * 32              # [P, 2, HID]
BOFF_W2 = BOFF_W1 + P * 2 * HID           # [P, 2, OUT_CH]
NBLOB_BF = BOFF_W2 + P * 2 * OUT_CH


def _log_cpb_np(h, w, w1, b1, w2):
    """Host fp32 mirror of the reference CPB MLP, on the 47x47 delta grid."""
    dy = np.arange(-(h - 1), h, dtype=np.float32)
    dx = np.arange(-(w - 1), w, dtype=np.float32)
    rel = np.stack(np.meshgrid(dy, dx, indexing="ij"), axis=-1)     # (2h-1, 2w-1, 2)
    denom = np.array([max(h - 1, 1), max(w - 1, 1)], dtype=np.float32)
    rel = rel / denom * np.float32(8.0)
    rel = np.sign(rel) * np.log2(np.float32(1.0) + np.abs(rel)) / np.float32(np.log2(8.0))
    hid_act = np.maximum(rel @ w1.T + b1, np.float32(0.0))          # (2h-1, 2w-1, c)
    tab = (hid_act @ w2.T).astype(np.float32)                       # (2h-1, 2w-1, heads)
    yl = np.repeat(np.arange(h), w)
    xl = np.tile(np.arange(w), h)
    DY = yl[:, None] - yl[None, :] + (h - 1)                        # (L, L)
    DX = xl[:, None] - xl[None, :] + (w - 1)
    return tab[DY, DX].transpose(2, 0, 1)                           # (heads, L, L)


def _prep_aux(qk_w, v_w, cpb_w1, cpb_b1, cpb_w2, sa_bias, mlp_w1, mlp_b1, mlp_w2, mlp_b2):
    scale = np.float32(1.0 / np.sqrt(QKD))
    qkwT = np.ascontiguousarray(qk_w.T).astype(np.float32).copy()   # (c, 2c)
    qkwT[:, :C] *= scale                                            # fold attn scale into q
    vwT = np.ascontiguousarray(v_w.T).astype(np.float32)            # (c, hid)

    bias = _log_cpb_np(HH, WW, cpb_w1, cpb_b1, cpb_w2)              # (heads, L, L)
    # multiplicative bias: exp(s+b) = exp(s)*exp(b); padded m-rows get 0 so
    # they vanish from the softmax sums and the attn@v contraction
    ebias = np.zeros((HEADS, LP, L), dtype=np.float32)
    ebias[:, :L, :] = np.exp(bias.transpose(0, 2, 1))               # [h, m, l]
    biasT_sb = np.empty((P, NCH, HEADS * L), dtype=np.float32)
    for ch in range(NCH):
        for h in range(HEADS):
            biasT_sb[:, ch, h * L:(h + 1) * L] = ebias[h, ch * P:(ch + 1) * P, :]

    w1T = np.empty((P, 2, HID), dtype=np.float32)                   # [p, kt, o]
    for kt in range(2):
        w1T[:, kt, :] = mlp_w1[:, kt * P:(kt + 1) * P].T
    w2T = np.empty((P, 2, OUT_CH), dtype=np.float32)
    for kt in range(2):
        w2T[:, kt, :] = mlp_w2[:, kt * P:(kt + 1) * P].T
    b1c = (mlp_w1 @ sa_bias.reshape(-1) + mlp_b1).astype(np.float32).reshape(2, P).T.copy()  # (128, 2)
    b2c = mlp_b2.astype(np.float32).reshape(P, 1).copy()
    return {
        "qkwT": qkwT, "vwT": vwT, "biasT": biasT_sb,
        "w1T": w1T, "w2T": w2T, "b1c": b1c, "b2c": b2c,
    }


def _pack_blobs(x, aux):
    """fp32 blob: x + projection weights; bf16 blob: exp-bias, ones, MLP."""
    blob = np.empty(NBLOB, dtype=np.float32)
    blob[OFF_X:OFF_X + NX] = np.ascontiguousarray(x, dtype=np.float32).reshape(-1)
    for off, name in ((OFF_QKW, "qkwT"), (OFF_VW, "vwT"),
                      (OFF_B1, "b1c"), (OFF_B2, "b2c")):
        a = aux[name].reshape(-1)
        blob[off:off + a.size] = a
    blob_bf = np.empty(NBLOB_BF, dtype=BF)
    blob_bf[BOFF_BIAS:BOFF_ONES] = aux["biasT"].reshape(-1).astype(BF)
    blob_bf[BOFF_ONES:BOFF_W1] = np.ones(P * 32, dtype=BF)
    blob_bf[BOFF_W1:BOFF_W2] = aux["w1T"].reshape(-1).astype(BF)
    blob_bf[BOFF_W2:] = aux["w2T"].reshape(-1).astype(BF)
    return blob, blob_bf


def _gap(ap):
    """View a (128, 1024) psum tile as (128, 2, 288): pieces at [0:288], [512:800]."""
    return ap.rearrange("p (g c) -> p g c", c=512)[:, :, :PIECE]


def _pieces(ap576):
    """View a contiguous (128, 576) AP as (128, 2, 288)."""
    return ap576.rearrange("p (g c) -> p g c", c=PIECE)


def build_kernel(seqs=SEQ_PER_CORE, num_devices=NCORES, repeat=1, skip=()):
    nc = bacc.Bacc("TRN2", target_bir_lowering=False, debug=False,
                   num_devices=num_devices)
    blob_d = nc.dram_tensor("blob", [NBLOB], F32R, kind="ExternalInput").ap()
    blobb_d = nc.dram_tensor("blobb", [NBLOB_BF], BF16, kind="ExternalInput").ap()
    y_d = nc.dram_tensor("y", [seqs, OUT_CH, L], F32, kind="ExternalOutput").ap()

    x_d = blob_d[OFF_X:OFF_X + seqs * SEQ_ELEMS].rearrange(
        "(t c l) -> t c l", c=C, l=L)
    qkwT_d = blob_d[OFF_QKW:OFF_VW].rearrange("(c o) -> c o", o=2 * C)
    vwT_d = blob_d[OFF_VW:OFF_B1].rearrange("(c o) -> c o", o=HID)
    b1c_d = blob_d[OFF_B1:OFF_B2].rearrange("(p k) -> p k", k=2).bitcast(F32)
    b2c_d = blob_d[OFF_B2:NBLOB].rearrange("(p k) -> p k", k=1).bitcast(F32)
    biasT_d = blobb_d[BOFF_BIAS:BOFF_ONES].rearrange(
        "(p n hl) -> p n hl", n=NCH, hl=HEADS * L)
    ones_d = blobb_d[BOFF_ONES:BOFF_W1].rearrange("(p k) -> p k", k=32)
    w1T_d = blobb_d[BOFF_W1:BOFF_W2].rearrange("(p k o) -> p k o", k=2, o=HID)
    w2T_d = blobb_d[BOFF_W2:].rearrange("(p k o) -> p k o", k=2, o=OUT_CH)

    EXP = mybir.ActivationFunctionType.Exp
    GELU = mybir.ActivationFunctionType.Gelu
    MULT = mybir.AluOpType.mult

    ngroups = (seqs + GROUP - 1) // GROUP

    with tile.TileContext(nc) as tc:
        with (
            tc.tile_pool(name="const", bufs=1) as cpool,
            tc.tile_pool(name="xin", bufs=3) as xpool,
            tc.tile_pool(name="qk", bufs=3) as qkpool,
            tc.tile_pool(name="vt", bufs=3) as vtpool,
            tc.tile_pool(name="ex", bufs=2) as expool,
            tc.tile_pool(name="sm", bufs=2) as smpool,
            tc.tile_pool(name="xb", bufs=2) as xbpool,
            tc.tile_pool(name="drb", bufs=2, space="DRAM") as drpool,
            tc.tile_pool(name="acts", bufs=GROUP) as apool,
            tc.tile_pool(name="mlp", bufs=2) as mpool,
            tc.tile_pool(name="psg", bufs=2, space="PSUM") as psg,
            tc.tile_pool(name="ps1", bufs=4, space="PSUM") as ps1,
        ):
            qkwT = cpool.tile([C, 2 * C], F32R)
            vwT = cpool.tile([C, HID], F32R)
            biasT = cpool.tile([P, NCH, HEADS * L], BF16)
            ones32 = cpool.tile([P, 32], BF16)
            w1T = cpool.tile([P, 2, HID], BF16)
            w2T = cpool.tile([P, 2, OUT_CH], BF16)
            b1c = cpool.tile([P, 2], F32)
            b2c = cpool.tile([P, 1], F32)
            for sb_t, dr in ((qkwT, qkwT_d), (vwT, vwT_d), (biasT, biasT_d),
                             (ones32, ones_d), (w1T, w1T_d), (w2T, w2T_d),
                             (b1c, b1c_d), (b2c, b2c_d)):
                nc.sync.dma_start(sb_t[:], dr[:])

            _last_exp = [None]
            _last_gelu = [None]
            # repeat>1 is a timing-only mode: reps chain serially through pools
            for _rep in range(repeat):
              for grp in range(ngroups):
                t0 = grp * GROUP
                gseqs = range(t0, min(t0 + GROUP, seqs))
                a_tiles = {}
                # ---------------- Phase A: attention ----------------
                for t in gseqs:
                    xt = xpool.tile([C, LP], F32R)
                    nc.sync.dma_start(xt[:, :L], x_d[t])
                    nc.gpsimd.memset(xt[:, L:].bitcast(F32), 0.0)

                    # qk projection: q rows (h0 d, h1 d), k rows (h0 d, h1 d)
                    q_sb = qkpool.tile([P, L], F32R, tag="q")
                    k_sb = qkpool.tile([P, LP], F32R, tag="k")
                    for mt, dst in ((0, q_sb[:, :L]), (1, k_sb[:, :L])):
                        pqk = psg.tile([P, 1024], F32, tag="g")
                        for lh in range(2):
                            nc.tensor.matmul(
                                _gap(pqk[:])[:, lh],
                                qkwT[:, mt * P:(mt + 1) * P],
                                xt[:, lh * PIECE:(lh + 1) * PIECE],
                                start=True, stop=True)
                        nc.vector.tensor_copy(_pieces(dst), _gap(pqk[:]))
                    nc.gpsimd.memset(k_sb[:, L:].bitcast(F32), 0.0)

                    # vT: (m-part chunks, 256 = both heads' d), bf16
                    vt_sb = vtpool.tile([P, NCH, HID], BF16)
                    for ch in range(NCH):
                        pv = ps1.tile([P, PIECE], F32, tag="s")
                        nc.tensor.matmul(pv[:, :HID], xt[:, ch * P:(ch + 1) * P], vwT[:],
                                         start=True, stop=True)
                        nc.vector.tensor_copy(vt_sb[:, ch, :], pv[:, :HID])

                    # scores + exp (bf16) per (chunk, head); bias-mult per chunk
                    ex_sb = expool.tile([P, NCH, HEADS * L], BF16)
                    for ch in range(NCH):
                        for h in range(HEADS):
                            # K=64 matmuls; the two heads run on disjoint PE
                            # row-groups into different PSUM banks (concurrent)
                            psc = psg.tile([P, 1024], F32, tag="g")
                            for lh in range(2):
                                nc.tensor.matmul(
                                    _gap(psc[:])[:, lh],
                                    k_sb[h * QKD:(h + 1) * QKD, ch * P:(ch + 1) * P],
                                    q_sb[h * QKD:(h + 1) * QKD, lh * PIECE:(lh + 1) * PIECE],
                                    start=True, stop=True)
                            e = nc.scalar.activation(
                                _pieces(ex_sb[:, ch, h * L:(h + 1) * L]), _gap(psc[:]), EXP)
                            if _last_gelu[0] is not None:
                                # keep ACT exp->gelu strictly phase-ordered: the two
                                # live in different ACT table sets, and a mid-phase
                                # switch costs ~1.3us
                                add_dep_helper(_last_gelu[0].ins, e.ins, sync=False,
                                               reason="ACT table set phase order")
                                _last_gelu[0] = None
                            _last_exp[0] = e
                        if "gmul" not in skip:
                            # multiplicative position bias, both heads at once
                            # (bf16 all-SBUF -> DVE fast mode)
                            nc.vector.tensor_tensor(
                                ex_sb[:, ch, :], ex_sb[:, ch, :],
                                biasT[:, ch, :], MULT)

                    # softmax denominators: ones-matmul into one PSUM row ->
                    # reciprocal -> broadcast row via DRAM round-trip
                    xb = xbpool.tile([P, 4 * PIECE], F32)
                    if "sums" not in skip:
                        inv_sb = smpool.tile([1, 4 * PIECE], F32, tag="inv")
                        for pc in range(4):
                            h, lh = pc // 2, pc % 2
                            pss = ps1.tile([P, PIECE], F32, tag="s")
                            for ch in range(NCH):
                                nc.tensor.matmul(
                                    pss[0:1, :], ones32[:, 0:1],
                                    ex_sb[:, ch, h * L + lh * PIECE: h * L + (lh + 1) * PIECE],
                                    start=(ch == 0), stop=(ch == NCH - 1))
                            nc.vector.reciprocal(
                                inv_sb[:, pc * PIECE:(pc + 1) * PIECE], pss[0:1, :])
                        inv_dr = drpool.tile([1, 4 * PIECE], F32)
                        nc.sync.dma_start(inv_dr[:], inv_sb[:])
                        nc.sync.dma_start(xb[:], inv_dr[:].to_broadcast((P, 4 * PIECE)))

                    # out = vT.T @ expT, normalized (DVE)
                    a_sb = apool.tile([P, 2, L], BF16)
                    for h in range(HEADS):
                        pms = [ps1.tile([P, PIECE], F32, tag="s", name=f"pm{_i}") for _i in range(2)]
                        for ch in range(NCH):
                            for lh in range(2):  # reuse loaded vT weights across both pieces
                                nc.tensor.matmul(
                                    pms[lh][:], vt_sb[:, ch, h * VD:(h + 1) * VD],
                                    ex_sb[:, ch, h * L + lh * PIECE: h * L + (lh + 1) * PIECE],
                                    start=(ch == 0), stop=(ch == NCH - 1))
                        for lh in range(2):
                            if "norm" in skip:
                                nc.vector.tensor_copy(
                                    a_sb[:, h, lh * PIECE:(lh + 1) * PIECE], pms[lh][:])
                            else:
                                nc.vector.tensor_tensor(
                                    a_sb[:, h, lh * PIECE:(lh + 1) * PIECE], pms[lh][:],
                                    xb[:, (h * 2 + lh) * PIECE:(h * 2 + lh + 1) * PIECE], MULT)
                    a_tiles[t] = a_sb

                # ---------------- Phase B: MLP ----------------
                for t in ([] if "mlp" in skip else gseqs):
                    a_sb = a_tiles[t]
                    g_sb = mpool.tile([P, 2, L], BF16, tag="g")
                    for mt in range(2):
                        py1 = psg.tile([P, 1024], F32, tag="g")
                        for lh in range(2):
                            for kt in range(2):
                                nc.tensor.matmul(
                                    _gap(py1[:])[:, lh],
                                    w1T[:, kt, mt * P:(mt + 1) * P],
                                    a_sb[:, kt, lh * PIECE:(lh + 1) * PIECE],
                                    start=(kt == 0), stop=(kt == 1))
                        _g = nc.scalar.activation(
                            _pieces(g_sb[:, mt, :]), _gap(py1[:]), GELU,
                            bias=b1c[:, mt:mt + 1], scale=1.0)
                        if _last_exp[0] is not None:
                            add_dep_helper(_last_exp[0].ins, _g.ins, sync=False,
                                           reason="ACT table set phase order")
                            _last_exp[0] = None
                        _last_gelu[0] = _g
                    y_sb = mpool.tile([OUT_CH, L], F32, tag="y")
                    for lh in range(2):
                        py2 = ps1.tile([P, PIECE], F32, tag="s")
                        for kt in range(2):
                            nc.tensor.matmul(
                                py2[:], w2T[:, kt, :],
                                g_sb[:, kt, lh * PIECE:(lh + 1) * PIECE],
                                start=(kt == 0), stop=(kt == 1))
                        nc.vector.tensor_scalar_add(
                            y_sb[:, lh * PIECE:(lh + 1) * PIECE], py2[:], b2c[:, 0:1])
                    nc.sync.dma_start(y_d[t], y_sb[:])
                if "mlp" in skip:
                    for t in gseqs:
                        nc.sync.dma_start(y_d[t].rearrange("c l -> c l"),
                                          a_tiles[t][:, 0, :].bitcast(F32))
    nc.compile()
    return nc


_CACHED = {}


def _get_nc():
    if "nc" not in _CACHED:
        _CACHED["nc"] = build_kernel()
    return _CACHED["nc"]


def make_in_maps(x, aux):
    xr = np.ascontiguousarray(x.reshape(NSEQ, C, L), dtype=np.float32)
    in_maps = []
    for i in range(NCORES):
        xs = xr[i * SEQ_PER_CORE:(i + 1) * SEQ_PER_CORE]
        blob, blob_bf = _pack_blobs(xs, aux)
        in_maps.append({"blob": blob, "blobb": blob_bf})
    return in_maps


def kernel(x, qk_w, v_w, cpb_w1, cpb_b1, cpb_w2, sa_bias,
           mlp_w1, mlp_b1, mlp_w2, mlp_b2):
    x = np.asarray(x)
    aux = _prep_aux(np.asarray(qk_w), np.asarray(v_w), np.asarray(cpb_w1),
                    np.asarray(cpb_b1), np.asarray(cpb_w2), np.asarray(sa_bias),
                    np.asarray(mlp_w1), np.asarray(mlp_b1), np.asarray(mlp_w2),
                    np.asarray(mlp_b2))
    nc = _get_nc()
    in_maps = make_in_maps(x, aux)
    res = run_bass_kernel_spmd(nc, in_maps, core_ids=list(range(NCORES)))
    y = np.concatenate([res.results[i]["y"] for i in range(NCORES)], axis=0)
    return y.reshape(B, S, OUT_CH, HH, WW)


if __name__ == "__main__":
    import reference
    inputs = reference.setup_inputs()
    inputs = {k: np.asarray(v) for k, v in inputs.items()}
    out = kernel(**inputs)
    exp = np.asarray(reference.reference(**reference.setup_inputs()))
    err = np.abs(out - exp).max() / np.abs(exp).max()
    print("Relative error:", err)
